# revision 17
# baseline (speedup 1.0000x reference)
"""MAE3D forward kernel for Trainium2 (Bass/Tile), data-parallel over batch.

Strategy:
  - Host does: patchify (pure reshape), argsort(noise), gather of visible
    patches, permutation bookkeeping, output unpermute.  The device runs a
    pure dense transformer on a permuted token order (attention is
    permutation-equivariant, so no gather/scatter on device).
  - Device layout is feature-major: activations live as x^T [E, S] so every
    projection is matmul(out=W@xT) with E contracted on partitions.
  - Attention uses transposed scores s^T[kj, qi] = k^T.T @ q^T per head
    (K=32 matmuls, 4 heads packed via tile_position row strips), exp runs
    directly PSUM->SBUF on ScalarE, and AV + softmax denominator accumulate
    in PSUM via col-strip-packed matmuls (denominator = ones-vector matmul).
  - One sample per core, 4 cores (B=4).
"""

import os

import numpy as np

import concourse.bass as bass
import concourse.mybir as mybir
import concourse.tile as tile
from concourse import bacc
from concourse.bass_utils import run_bass_kernel_spmd

FP = mybir.dt.float32
AF = mybir.ActivationFunctionType
ALU = mybir.AluOpType
AX = mybir.AxisListType

PATCH = 5
G = 60
E = 256
H = 8
DH = 32
NPD = G // PATCH
NP = NPD ** 3            # 1728
PC = PATCH ** 3          # 125
LE, LD = 4, 2
NCLS = 40
B = 4
NMASK = 1296
NVIS = NP - NMASK        # 432
QC = 432                 # query chunk (free-dim of score matmuls)
QH = NP // 2             # decoder queries per core (TP2 split)
INV_SQRT_DH = float(1.0 / np.sqrt(DH))


def _blocks(total, step=128):
    return [(s, min(step, total - s)) for s in range(0, total, step)]


def _nz(a):
    return a is not None and not np.allclose(np.asarray(a), 0.0)


class _LayerW:
    """Per-layer weights, pre-transposed host-side for lhsT use."""

    def __init__(self, nc, pfx, i, qkv_w, qkv_b, out_w, out_b, ln1w, ln1b,
                 ln2w, ln2b, f1w, f1b, f2w, f2b):
        f32 = np.float32
        self.qkvT = nc.inline_tensor(np.ascontiguousarray(qkv_w.T, f32),
                                     name=f"{pfx}{i}_qkvT")       # [256, 768]
        self.outT = nc.inline_tensor(np.ascontiguousarray(out_w.T, f32),
                                     name=f"{pfx}{i}_outT")       # [256, 256]
        self.f1T = nc.inline_tensor(np.ascontiguousarray(f1w.T, f32),
                                    name=f"{pfx}{i}_f1T")         # [256, 1024]
        self.f2T = nc.inline_tensor(np.ascontiguousarray(f2w.T, f32),
                                    name=f"{pfx}{i}_f2T")         # [1024, 256]
        self.qkv_b = np.asarray(qkv_b, f32)
        self.out_b = np.asarray(out_b, f32)
        self.f1b = np.asarray(f1b, f32)
        self.f2b = np.asarray(f2b, f32)
        self.ln1 = (np.asarray(ln1w, f32), np.asarray(ln1b, f32))
        self.ln2 = (np.asarray(ln2w, f32), np.asarray(ln2b, f32))
        self.qkv_b_t = (nc.inline_tensor(self.qkv_b.reshape(768, 1),
                                         name=f"{pfx}{i}_qkvb")
                        if _nz(self.qkv_b) else None)
        self.vb_t = (nc.inline_tensor(self.qkv_b[512:].reshape(1, 256),
                                      name=f"{pfx}{i}_vb")
                     if _nz(self.qkv_b) else None)
        self.f1b_t = (nc.inline_tensor(self.f1b.reshape(1024, 1),
                                       name=f"{pfx}{i}_f1b")
                      if _nz(self.f1b) else None)
        # out_b / f2b / ln affine are zeros/ones in this model; assert so a
        # silently different harness setup fails loudly instead of wrong.
        assert not _nz(self.out_b) and not _nz(self.f2b)
        assert np.allclose(self.ln1[0], 1) and not _nz(self.ln1[1])
        assert np.allclose(self.ln2[0], 1) and not _nz(self.ln2[1])


def _build(weights):
    """Build the bass program. Returns nc."""
    ndev = 1 if os.environ.get("KERNEL_TLSIM") == "1" else 2 * B
    nc = bacc.Bacc("TRN2", target_bir_lowering=False, debug=False,
                   num_devices=ndev)

    # ---- I/O ----
    xgvT_d = nc.dram_tensor("xgvT", [128, NVIS], FP, kind="ExternalInput")
    posT_d = nc.dram_tensor("posT", [E, NVIS], FP, kind="ExternalInput")
    dppT_d = nc.dram_tensor("dppT", [E, NP], FP, kind="ExternalInput")
    sel_d = nc.dram_tensor("sel", [128, 2], FP, kind="ExternalInput")
    predT_d = nc.dram_tensor("predT", [PC, QH], FP, kind="ExternalOutput")
    cls_d = nc.dram_tensor("cls", [NCLS, 1], FP, kind="ExternalOutput")

    # ---- inline weights ----
    f32 = np.float32
    pw = np.asarray(weights["patch_w"], f32).reshape(E, PC)
    pwT = np.zeros((128, E), f32)
    pwT[:PC, :] = pw.T
    pwT_t = nc.inline_tensor(pwT, name="w_patchT")
    predT_w = nc.inline_tensor(
        np.ascontiguousarray(np.asarray(weights["pred_w"], f32).T, f32),
        name="w_predT")                                        # [256, 125]
    clsT_w = nc.inline_tensor(
        np.ascontiguousarray(np.asarray(weights["cls_w"], f32).T, f32),
        name="w_clsT")                                         # [256, 40]
    pred_b = np.asarray(weights["pred_b"], f32)
    cls_b = np.asarray(weights["cls_b"], f32)
    pred_b_t = (nc.inline_tensor(pred_b.reshape(PC, 1), name="pred_b")
                if _nz(pred_b) else None)
    cls_b_t = (nc.inline_tensor(cls_b.reshape(NCLS, 1), name="cls_b")
               if _nz(cls_b) else None)

    enc_lw = [_LayerW(nc, "enc", i, *(weights[f"enc_{k}"][i] for k in (
        "qkv_w", "qkv_b", "out_w", "out_b", "ln1_w", "ln1_b", "ln2_w",
        "ln2_b", "ffn1_w", "ffn1_b", "ffn2_w", "ffn2_b"))) for i in range(LE)]
    dec_lw = [_LayerW(nc, "dec", i, *(weights[f"dec_{k}"][i] for k in (
        "qkv_w", "qkv_b", "out_w", "out_b", "ln1_w", "ln1_b", "ln2_w",
        "ln2_b", "ffn1_w", "ffn1_b", "ffn2_w", "ffn2_b"))) for i in range(LD)]

    with tile.TileContext(nc) as tc:
        import contextlib
        ctx = contextlib.ExitStack()
        with ctx:
            singles = ctx.enter_context(tc.tile_pool(name="singles", bufs=1))
            wpool = ctx.enter_context(tc.tile_pool(name="wpool", bufs=1))
            xpool = ctx.enter_context(tc.tile_pool(name="xpool", bufs=1))
            act = ctx.enter_context(tc.tile_pool(name="act", bufs=1))
            expp = ctx.enter_context(tc.tile_pool(name="expp", bufs=2))
            hp = ctx.enter_context(tc.tile_pool(name="hp", bufs=2))
            misc = ctx.enter_context(tc.tile_pool(name="misc", bufs=2))
            outp = ctx.enter_context(tc.tile_pool(name="outp", bufs=2))
            ps_s = ctx.enter_context(
                tc.tile_pool(name="ps_s", bufs=1, space="PSUM"))
            ps_a = ctx.enter_context(
                tc.tile_pool(name="ps_a", bufs=2, space="PSUM"))
            ps_m = ctx.enter_context(
                tc.tile_pool(name="ps_m", bufs=2, space="PSUM"))

            ones = singles.tile([128, 128], FP)
            nc.vector.memset(ones, 1.0)
            eps = singles.tile([128, 1], FP)
            nc.vector.memset(eps, 1e-5)

            def ln(xT, S):
                """In-place LayerNorm over E (partition dim) of xT (2 tiles)."""
                for q0, qs in _blocks(S, QC):
                    sq = misc.tile([128, 2, QC], FP, tag="sq")
                    for k in range(2):
                        nc.vector.tensor_mul(sq[:, k, :qs],
                                             xT[k][:, q0:q0 + qs],
                                             xT[k][:, q0:q0 + qs])
                    st1 = ps_m.tile([128, 512], FP, tag="mm")
                    for k in range(2):
                        nc.tensor.matmul(st1[:, :qs], ones,
                                         xT[k][:, q0:q0 + qs],
                                         start=(k == 0), stop=(k == 1))
                    st2 = ps_m.tile([128, 512], FP, tag="mm")
                    for k in range(2):
                        nc.tensor.matmul(st2[:, :qs], ones,
                                         sq[:, k, :qs],
                                         start=(k == 0), stop=(k == 1))
                    mt = misc.tile([128, QC], FP, tag="mt")
                    rt = misc.tile([128, QC], FP, tag="rt")
                    nc.vector.tensor_scalar_mul(mt[:, :qs], st1[:, :qs],
                                                1.0 / E)          # mean (repl)
                    nc.vector.tensor_scalar_mul(rt[:, :qs], st2[:, :qs],
                                                1.0 / E)          # E[x^2]
                    sqm = misc.tile([128, QC], FP, tag="sqm")
                    nc.vector.tensor_mul(sqm[:, :qs], mt[:, :qs], mt[:, :qs])
                    nc.vector.tensor_tensor(rt[:, :qs], rt[:, :qs],
                                            sqm[:, :qs], ALU.subtract)
                    nc.scalar.activation(rt[:, :qs], rt[:, :qs], AF.Sqrt,
                                         bias=eps)
                    nc.vector.reciprocal(rt[:, :qs], rt[:, :qs])
                    for k in range(2):
                        nc.vector.tensor_tensor(
                            xT[k][:, q0:q0 + qs], xT[k][:, q0:q0 + qs],
                            mt[:, :qs], ALU.subtract)
                        nc.vector.tensor_tensor(
                            xT[k][:, q0:q0 + qs], xT[k][:, q0:q0 + qs],
                            rt[:, :qs], ALU.mult)

            def layer(xT, S, lw, tag, Q=None):
                """One post-norm transformer layer, in place on xT.

                Q restricts the query side (attention rows, FFN/LN/residual
                columns) to xT[:, :Q]; K/V always cover all S tokens."""
                if Q is None:
                    Q = S
                kjb = _blocks(S)           # key blocks (partition dim)
                qcb = _blocks(Q, QC)       # query chunks (free dim)
                wq3 = wpool.tile([128, 2, 768], FP, tag="wqkv", name="wqkv")
                wo3 = wpool.tile([128, 2, 256], FP, tag="wout", name="wout")
                w13 = wpool.tile([128, 2, 1024], FP, tag="wf1", name="wf1")
                w23 = wpool.tile([128, 8, 256], FP, tag="wf2", name="wf2")
                nc.sync.dma_start(wq3, lw.qkvT.rearrange("(a p) o -> p a o",
                                                         p=128))
                nc.sync.dma_start(wo3, lw.outT.rearrange("(a p) o -> p a o",
                                                         p=128))
                nc.sync.dma_start(w13, lw.f1T.rearrange("(a p) o -> p a o",
                                                        p=128))
                nc.sync.dma_start(w23, lw.f2T.rearrange("(a p) o -> p a o",
                                                        p=128))
                wq = [wq3[:, k] for k in range(2)]
                wo = [wo3[:, k] for k in range(2)]
                w1 = [w13[:, k] for k in range(2)]
                w2 = [w23[:, k] for k in range(8)]
                qkvb = vbias = None
                if lw.qkv_b_t is not None:
                    qkvb = misc.tile([128, 6], FP, tag="qkvb")
                    nc.sync.dma_start(
                        qkvb, lw.qkv_b_t.rearrange("(a p) o -> p (a o)", p=128))
                    vb1 = misc.tile([1, 256], FP, tag="vb1")
                    nc.sync.dma_start(vb1, lw.vb_t[:, :])
                    vps = ps_m.tile([128, 512], FP, tag="mm")
                    nc.tensor.matmul(vps[:, :256], ones[0:1, :], vb1)
                    vbias = misc.tile([128, 256], FP, tag="vbias")
                    nc.vector.tensor_copy(vbias, vps[:, :256])
                f1b = None
                if lw.f1b_t is not None:
                    f1b = misc.tile([128, 8], FP, tag="f1b")
                    nc.sync.dma_start(
                        f1b, lw.f1b_t.rearrange("(a p) o -> p (a o)", p=128))

                # qT/kT feature-major [256, S]; v token-major [S, 256]
                qkT = [act.tile([128, S], FP, tag=f"qk{m}_{tag}", name=f"qk{m}_{tag}")
                       for m in range(4)]
                for m in range(4):
                    for q0, qs in (qcb if m < 2 else _blocks(S, QC)):
                        ps = ps_m.tile([128, 512], FP, tag="mm")
                        for k in range(2):
                            nc.tensor.matmul(
                                ps[:, :qs], wq[k][:, 128 * m:128 * (m + 1)],
                                xT[k][:, q0:q0 + qs],
                                start=(k == 0), stop=(k == 1))
                        if qkvb is not None:
                            nc.vector.tensor_scalar_add(
                                qkT[m][:, q0:q0 + qs], ps[:, :qs],
                                scalar1=qkvb[:, m:m + 1])
                        else:
                            nc.vector.tensor_copy(qkT[m][:, q0:q0 + qs],
                                                  ps[:, :qs])
                vt = act.tile([128, len(kjb), 256], FP, tag=f"v_{tag}")
                for bi, (t0, ts) in enumerate(kjb):
                    ps = ps_m.tile([128, 512], FP, tag="mm")
                    for k in range(2):
                        nc.tensor.matmul(ps[:ts, :256],
                                         xT[k][:, t0:t0 + ts],
                                         wq[k][:, 512:768],
                                         start=(k == 0), stop=(k == 1))
                    if vbias is not None:
                        nc.vector.tensor_tensor(
                            vt[:ts, bi, :], ps[:ts, :256],
                            vbias[:ts], ALU.add)
                    else:
                        nc.vector.tensor_copy(vt[:ts, bi, :], ps[:ts, :256])

                oT = [act.tile([128, S], FP, tag=f"oT{g}_{tag}", name=f"oT{g}_{tag}")
                      for g in range(2)]
                for g in range(2):
                    for q0, qs in qcb:
                        ot = ps_a.tile([128, 512], FP, tag="acc")
                        den = ps_a.tile([128, 512], FP, tag="acc")
                        nb = len(kjb)
                        for bi, (t0, ts) in enumerate(kjb):
                            sc = ps_s.tile([128, 4, 512], FP, tag="scores")
                            for h in range(4):
                                nc.tensor.matmul(
                                    sc[:ts, h, :qs],
                                    qkT[2 + g][32 * h:32 * h + 32,
                                               t0:t0 + ts],
                                    qkT[g][32 * h:32 * h + 32, q0:q0 + qs],
                                    tile_position=(32 * h, 0))
                            ex = expp.tile([128, 4, 512], FP, tag="exp")
                            nc.scalar.activation(ex[:ts, :, :qs],
                                                 sc[:ts, :, :qs], AF.Exp,
                                                 scale=INV_SQRT_DH)
                            for h in range(4):
                                hc = 32 * (4 * g + h)
                                nc.tensor.matmul(
                                    ot[32 * h:32 * h + 32, :qs],
                                    vt[:ts, bi, hc:hc + 32],
                                    ex[:ts, h, :qs],
                                    start=(bi == 0), stop=(bi == nb - 1),
                                    tile_position=(0, 32 * h))
                                nc.tensor.matmul(
                                    den[32 * h:32 * h + 32, :qs],
                                    ones[:ts, :32], ex[:ts, h, :qs],
                                    start=(bi == 0), stop=(bi == nb - 1),
                                    tile_position=(0, 32 * h))
                        rec = misc.tile([128, QC], FP, tag="rec")
                        nc.vector.reciprocal(rec[:, :qs], den[:, :qs])
                        nc.vector.tensor_tensor(
                            oT[g][:, q0:q0 + qs], ot[:, :qs],
                            rec[:, :qs], ALU.mult)

                # out-proj + residual
                for m in range(2):
                    for q0, qs in qcb:
                        ps = ps_m.tile([128, 512], FP, tag="mm")
                        for k in range(2):
                            nc.tensor.matmul(
                                ps[:, :qs], wo[k][:, 128 * m:128 * (m + 1)],
                                oT[k][:, q0:q0 + qs],
                                start=(k == 0), stop=(k == 1))
                        nc.vector.tensor_add(xT[m][:, q0:q0 + qs],
                                             xT[m][:, q0:q0 + qs],
                                             ps[:, :qs])
                ln(xT, Q)
                # FFN
                for q0, qs in qcb:
                    ht = hp.tile([128, 8, QC], FP, tag="hT")
                    for m in range(8):
                        ps = ps_m.tile([128, 512], FP, tag="mm")
                        for k in range(2):
                            nc.tensor.matmul(
                                ps[:, :qs], w1[k][:, 128 * m:128 * (m + 1)],
                                xT[k][:, q0:q0 + qs],
                                start=(k == 0), stop=(k == 1))
                        nc.scalar.activation(
                            ht[:, m, :qs], ps[:, :qs], AF.Gelu,
                            bias=(f1b[:, m:m + 1] if f1b is not None else 0.0))
                    for m in range(2):
                        ps = ps_m.tile([128, 512], FP, tag="mm")
                        for k in range(8):
                            nc.tensor.matmul(
                                ps[:, :qs], w2[k][:, 128 * m:128 * (m + 1)],
                                ht[:, k, :qs],
                                start=(k == 0), stop=(k == 7))
                        nc.vector.tensor_add(xT[m][:, q0:q0 + qs],
                                             xT[m][:, q0:q0 + qs],
                                             ps[:, :qs])
                ln(xT, Q)

            # ---- patch embed (visible tokens) ----
            xgv = xpool.tile([128, NVIS], FP, tag="xgv")
            nc.sync.dma_start(xgv, xgvT_d[:, :])
            wpt = xpool.tile([128, E], FP, tag="wpt")
            nc.sync.dma_start(wpt, pwT_t[:, :])
            xTe = [xpool.tile([128, NVIS], FP, tag=f"xTe{k}", name=f"xTe{k}") for k in range(2)]
            for k in range(2):
                pos = misc.tile([128, NVIS], FP, tag="pos")
                nc.sync.dma_start(pos, posT_d[128 * k:128 * (k + 1), :])
                ps = ps_m.tile([128, 512], FP, tag="mm")
                nc.tensor.matmul(ps[:, :NVIS],
                                 wpt[:, 128 * k:128 * (k + 1)], xgv)
                nc.vector.tensor_add(xTe[k], ps[:, :NVIS], pos)

            for i in range(LE):
                layer(xTe, NVIS, enc_lw[i], "e")

            # ---- cls head ----
            wcl = misc.tile([128, 2, NCLS], FP, tag="wcls")
            nc.sync.dma_start(wcl,
                              clsT_w.rearrange("(a p) o -> p a o", p=128))
            mean = misc.tile([128, 2], FP, tag="mean")
            for k in range(2):
                nc.vector.reduce_sum(mean[:, k:k + 1], xTe[k], axis=AX.X)
            cps = ps_m.tile([128, 512], FP, tag="mm")
            for k in range(2):
                nc.tensor.matmul(cps[:NCLS, 0:1], wcl[:, k, :],
                                 mean[:, k:k + 1],
                                 start=(k == 0), stop=(k == 1))
            csb = misc.tile([NCLS, 1], FP, tag="csb")
            nc.vector.tensor_scalar_mul(csb, cps[:NCLS, 0:1], 1.0 / NVIS)
            if cls_b_t is not None:
                cbt = misc.tile([NCLS, 1], FP, tag="cbt")
                nc.sync.dma_start(cbt, cls_b_t[:, :])
                nc.vector.tensor_add(csb, csb, cbt)
            nc.sync.dma_start(cls_d[:, :], csb)

            # ---- decoder input ----
            sel = misc.tile([128, 2], FP, tag="sel")
            nc.sync.dma_start(sel, sel_d[:, :])
            xTd = [xpool.tile([128, NP], FP, tag=f"xTd{k}", name=f"xTd{k}") for k in range(2)]
            for k in range(2):
                nc.sync.dma_start(xTd[k], dppT_d[128 * k:128 * (k + 1), :])
                for w, c0 in ((0, 0), (1, QH)):
                    tmp = misc.tile([128, NVIS], FP, tag="encsel")
                    nc.vector.tensor_scalar_mul(tmp, xTe[k],
                                                scalar1=sel[:, w:w + 1])
                    nc.vector.tensor_add(xTd[k][:, c0:c0 + NVIS],
                                         xTd[k][:, c0:c0 + NVIS], tmp)

            drp = ctx.enter_context(
                tc.tile_pool(name="drp", bufs=1, space="DRAM"))
            for i in range(LD):
                layer(xTd, NP, dec_lw[i], "d", Q=QH)
                if i < LD - 1 and os.environ.get("KERNEL_TLSIM") != "1":
                    # exchange halves with pair partner: partner = sum - mine
                    ib = drp.tile([E, QH], FP, name=f"ib{i}")
                    ob = drp.tile([E, QH], FP, name=f"ob{i}")
                    for k in range(2):
                        nc.sync.dma_start(ib[128 * k:128 * (k + 1), :],
                                          xTd[k][:, :QH])
                    nc.gpsimd.collective_compute(
                        "AllReduce", mybir.AluOpType.add,
                        replica_groups=[[0, 1], [2, 3], [4, 5], [6, 7]],
                        ins=[ib[:, :]], outs=[ob[:, :]])
                    for k in range(2):
                        tsum = misc.tile([128, QH], FP, tag="tsum")
                        nc.sync.dma_start(tsum, ob[128 * k:128 * (k + 1), :])
                        nc.vector.tensor_tensor(xTd[k][:, QH:], tsum,
                                                xTd[k][:, :QH], ALU.subtract)

            # ---- pred head ----
            wpr = [xpool.tile([128, PC], FP, tag=f"wpr{k}", name=f"wpr{k}") for k in range(2)]
            for k in range(2):
                nc.sync.dma_start(wpr[k], predT_w[128 * k:128 * (k + 1), :])
            pbt = None
            if pred_b_t is not None:
                pbt = misc.tile([PC, 1], FP, tag="pbt")
                nc.sync.dma_start(pbt, pred_b_t[:, :])
            for q0, qs in _blocks(QH, QC):
                ps = ps_m.tile([128, 512], FP, tag="mm")
                for k in range(2):
                    nc.tensor.matmul(ps[:PC, :qs], wpr[k],
                                     xTd[k][:, q0:q0 + qs],
                                     start=(k == 0), stop=(k == 1))
                po = outp.tile([PC, QC], FP, tag="po")
                if pbt is not None:
                    nc.vector.tensor_scalar_add(po[:, :qs], ps[:PC, :qs],
                                                scalar1=pbt)
                else:
                    nc.vector.tensor_copy(po[:, :qs], ps[:PC, :qs])
                nc.sync.dma_start(predT_d[:, q0:q0 + qs], po[:, :qs])

    nc.finalize()
    return nc


_CACHE = {}


def kernel(**inputs):
    inputs = {k: np.asarray(v, np.float32) for k, v in inputs.items()}
    x = inputs["x"]
    noise = inputs["noise"]

    # ---- host-side prep ----
    xg = x.reshape(B, 1, NPD, PATCH, NPD, PATCH, NPD, PATCH)
    xg = np.ascontiguousarray(
        xg.transpose(0, 2, 4, 6, 1, 3, 5, 7)).reshape(B, NP, PC)
    ids_shuffle = np.argsort(noise, axis=1, kind="stable")
    ids_mask = ids_shuffle[:, :NMASK]
    ids_keep = ids_shuffle[:, NMASK:]
    mask = np.zeros((B, NP), np.float32)
    np.put_along_axis(mask, ids_mask, 1.0, axis=1)

    pos = np.asarray(inputs["pos_embed"], np.float32)[0]          # [NP, E]
    dpos = np.asarray(inputs["dec_pos_embed"], np.float32)[0]     # [NP, E]
    mtok = np.asarray(inputs["mask_token"], np.float32).reshape(E)
    patch_b = np.asarray(inputs["patch_b"], np.float32)

    key = "prog"
    if key not in _CACHE:
        _CACHE[key] = _build(inputs)
    nc = _CACHE[key]

    in_maps = []
    perms = []
    for s in range(B):
        base = np.concatenate([ids_keep[s], ids_mask[s]])
        xgvT = np.zeros((128, NVIS), np.float32)
        xgvT[:PC, :] = xg[s][ids_keep[s]].T
        posT = np.ascontiguousarray((pos[ids_keep[s]] + patch_b).T)
        for r in range(2):
            perm = base if r == 0 else np.roll(base, -QH)
            perms.append(perm)
            dpp = dpos[perm].copy()
            msk = np.ones(NP, bool)
            inv = np.empty(NP, np.int64)
            inv[perm] = np.arange(NP)
            msk[inv[ids_keep[s]]] = False
            dpp[msk] += mtok
            dppT = np.ascontiguousarray(dpp.T)
            sel = np.zeros((128, 2), np.float32)
            sel[:, r] = 1.0
            in_maps.append({"xgvT": xgvT, "posT": posT, "dppT": dppT,
                            "sel": sel})

    res = run_bass_kernel_spmd(nc, in_maps, core_ids=list(range(2 * B)))

    pred = np.empty((B, NP, PC), np.float32)
    cls_logits = np.empty((B, NCLS), np.float32)
    for s in range(B):
        for r in range(2):
            c = 2 * s + r
            predT = res.results[c]["predT"]       # [PC, QH]
            pred[s, perms[c][:QH]] = predT.T
        cls_logits[s] = res.results[2 * s]["cls"].reshape(NCLS)

    return pred, xg, mask, cls_logits
